# revision 1
# baseline (speedup 1.0000x reference)
"""Bass/Trainium2 kernel for attention-LSTM decoder (nn_Attention_49289044688898).

Data-parallel over batch: 512 rows -> 8 NeuronCores x 64 rows. Weights replicated.
Within a core, the 64 rows are split into TWO interleaved groups of 32 so the
attention spine of one group overlaps the LSTM tail of the other.

Per group g, per decode step s (26 steps):
  q   = h @ Wh                                  (PE, bh folded into Hproj)
  arg = HprojT + qT (broadcast over t)          (DVE, bf16 2x)
  th  = tanh(arg)                               (ACT)
  e   = sum_h Ws[h]*th[h, t, b]                 (PE, Ws stationary, col-groups)
  alpha = softmax_t(e)                          (DVE/ACT small)
  ctx = sum_t alpha[b,t]*batch_H[b,t,c]         (PE, block-diag alpha stationary)
  z   = ctx @ Kc + h @ R + onehot @ Ko'         (PE; Ko' has lstm_bias folded)
  gates (sigmoid via 0.5*tanh(x/2)+0.5) / c / h (ACT/DVE)
  probs[:, s, :] = h @ Wgen + bgen              (PE)
Layouts (per group, GB=32 rows):
  attention world: [128 part = h_lo, 4 h_hi, 64 t, 32 b]   (h = h_hi*128 + h_lo)
  context world:   [128 part = (b%2)*64 + t, 16 kt=b//2, 512 c]
  LSTM world:      [32 part = b, free]
"""

import os
import numpy as np
import ml_dtypes
from contextlib import ExitStack

B, T, C, H, NCC, S = 512, 64, 512, 512, 96, 26
NCORES = 8
BS = B // NCORES          # 64 batch rows per core
NG = 2                    # groups per core
GB = BS // NG             # 32 rows per group
BF = ml_dtypes.bfloat16

_CACHE = {}


def build_bass():
    import concourse.bass as bass
    import concourse.bacc as bacc
    import concourse.tile as tile
    import concourse.mybir as mybir

    f32 = mybir.dt.float32
    bf16 = mybir.dt.bfloat16
    AF = mybir.ActivationFunctionType
    AX = mybir.AxisListType

    nc = bacc.Bacc("TRN2", target_bir_lowering=False)

    # ---- DRAM I/O ----
    # bHT: [g, c, t, b32] ; bHc: [g, kt, (b2 t), c]
    bHT_d = nc.dram_tensor("bHT", [NG, C, T, GB], bf16, kind="ExternalInput")
    bHc_d = nc.dram_tensor("bHc", [NG, GB // 2, 128, C], bf16, kind="ExternalInput")
    wi_d = nc.dram_tensor("wi", [C, H], bf16, kind="ExternalInput")
    wh_d = nc.dram_tensor("wh", [H, H], bf16, kind="ExternalInput")
    bh_d = nc.dram_tensor("bh", [128, 4], f32, kind="ExternalInput")
    ws_d = nc.dram_tensor("ws", [128, 4, 32], bf16, kind="ExternalInput")
    kc_d = nc.dram_tensor("kc", [C, 4 * H], bf16, kind="ExternalInput")
    rr_d = nc.dram_tensor("rr", [H, 4 * H], bf16, kind="ExternalInput")
    ko_d = nc.dram_tensor("ko", [NCC, 4 * H], bf16, kind="ExternalInput")
    oh_d = nc.dram_tensor("oh", [NCC, S, BS], bf16, kind="ExternalInput")
    wg_d = nc.dram_tensor("wg", [H, NCC], bf16, kind="ExternalInput")
    bg_d = nc.dram_tensor("bg", [BS, NCC], f32, kind="ExternalInput")
    out_d = nc.dram_tensor("out", [BS, S, NCC], f32, kind="ExternalOutput")
    escr_d = nc.dram_tensor("escr", [NG, S, T * GB], f32)  # scratch for e scatter

    NCH = T * GB // 512  # 4 (t,b)-chunks of 512 per group

    with tile.TileContext(nc) as tc, ExitStack() as ctx:
        big = ctx.enter_context(tc.tile_pool(name="big", bufs=1))
        wpool = ctx.enter_context(tc.tile_pool(name="wpool", bufs=1))
        small = ctx.enter_context(tc.tile_pool(name="small", bufs=2))
        tiny = ctx.enter_context(tc.tile_pool(name="tiny", bufs=4))
        gates = ctx.enter_context(tc.tile_pool(name="gates", bufs=4))
        state = ctx.enter_context(tc.tile_pool(name="state", bufs=2))
        pzg = [ctx.enter_context(tc.tile_pool(name=f"pz{g}", bufs=1, space="PSUM"))
               for g in range(NG)]
        peg = [ctx.enter_context(tc.tile_pool(name=f"pe{g}", bufs=1, space="PSUM"))
               for g in range(NG)]
        pzj = ctx.enter_context(tc.tile_pool(name="pzj", bufs=3, space="PSUM"))

        dma = nc.sync
        import concourse.bass as _b

        # ---- load weights / big tensors ----
        bHc = [big.tile([128, GB // 2, C], bf16, tag=f"bHc{g}", name=f"bHc{g}") for g in range(NG)]
        for g in range(NG):
            dma.dma_start(out=bHc[g], in_=bHc_d[g].rearrange("k p c -> p k c"))
        # batch_H^T (prolog only; shares slots with tanh buffers)
        bHT = [big.tile([128, 4, T * GB], bf16, tag=f"th{g}", name=f"bHT{g}") for g in range(NG)]
        for g in range(NG):
            dma.dma_start(
                out=bHT[g],
                in_=bHT_d[g].rearrange("(ch cl) t b -> cl ch (t b)", cl=128))

        wi = wpool.tile([128, 4, H], bf16, tag="wi")
        dma.dma_start(out=wi, in_=wi_d[:].rearrange("(ch cl) h -> cl ch h", cl=128))
        wh = wpool.tile([128, 4, H], bf16, tag="wh")
        dma.dma_start(out=wh, in_=wh_d[:].rearrange("(hh hl) h -> hl hh h", hl=128))
        bh = wpool.tile([128, 4], f32, tag="bh")
        dma.dma_start(out=bh, in_=bh_d[:])
        ws = wpool.tile([128, 4, 32], bf16, tag="ws")
        dma.dma_start(out=ws, in_=ws_d[:])
        kc = wpool.tile([128, 4, 4 * H], bf16, tag="kc")
        dma.dma_start(out=kc, in_=kc_d[:].rearrange("(kh kl) n -> kl kh n", kl=128))
        rr = wpool.tile([128, 4, 4 * H], bf16, tag="rr")
        dma.dma_start(out=rr, in_=rr_d[:].rearrange("(kh kl) n -> kl kh n", kl=128))
        ko = wpool.tile([NCC, 4 * H], bf16, tag="ko")
        dma.dma_start(out=ko, in_=ko_d[:])
        oh = wpool.tile([NCC, S, BS], bf16, tag="oh")
        dma.dma_start(out=oh, in_=oh_d[:])
        wg = wpool.tile([128, 4, NCC], bf16, tag="wg")
        dma.dma_start(out=wg, in_=wg_d[:].rearrange("(hh hl) n -> hl hh n", hl=128))
        bg = wpool.tile([BS, NCC], f32, tag="bg")
        dma.dma_start(out=bg, in_=bg_d[:])

        # block-diag alpha holders (zeroed once)
        ablk = [wpool.tile([128, GB // 2, GB], bf16, tag=f"ablk{g}", name=f"ablk{g}")
                for g in range(NG)]
        for g in range(NG):
            nc.vector.memset(ablk[g], 0.0)

        # initial state (joint across groups)
        hTj = [state.tile([128, 4, BS], bf16, tag="hT", name="hT0")]
        nc.vector.memset(hTj[0], 0.0)
        c_stj = [state.tile([BS, H], f32, tag="c", name="c0")]
        nc.vector.memset(c_stj[0], 0.0)
        alpha_pad = [None] * NG
        for g in range(NG):
            alpha_pad[g] = small.tile([GB, 128], bf16, tag=f"apad{g}", name=f"apad{g}")
            nc.vector.memset(alpha_pad[g], 0.0)

        # ---- prolog: HprojT[g] = (batch_H @ Wi)^T + bh ----
        hprojT = [big.tile([128, 4, T * GB], bf16, tag=f"hp{g}", name=f"hp{g}") for g in range(NG)]
        for g in range(NG):
            for m in range(4):
                for n in range(NCH):
                    ps = pzg[g].tile([128, 512], f32, tag=f"pz{g}")
                    for k in range(4):
                        nc.tensor.matmul(
                            ps,
                            wi[:, k, m * 128:(m + 1) * 128],
                            bHT[g][:, k, n * 512:(n + 1) * 512],
                            start=(k == 0), stop=(k == 3),
                        )
                    nc.scalar.activation(
                        out=hprojT[g][:, m, n * 512:(n + 1) * 512], in_=ps,
                        func=AF.Identity, bias=bh[:, m:m + 1], scale=1.0,
                    )

        def bcast_t(ap2):
            # [128, GB(b)] -> [128, T(t, step0), GB(b)]
            return _b.AP(tensor=ap2.tensor, offset=ap2.offset,
                         ap=[ap2.ap[0], [0, T], ap2.ap[1]])

        # joint-LSTM state: hT holds BOTH groups' columns [128, 4, 64]
        # q matmul is joint (M=64); attention is per-group (b-halves of qT cols)

        def phase_q(s):
            # q = h @ Wh for all 64 rows -> qT [128, 4, 64]
            pq = pzj.tile([BS, H], f32, tag="pzj")
            for k in range(4):
                nc.tensor.matmul(pq, hTj[0][:, k, :], wh[:, k, :],
                                 start=(k == 0), stop=(k == 3))
            q_sb = small.tile([BS, H], bf16, tag="q_sb", bufs=2)
            nc.scalar.copy(q_sb, pq)
            qT = small.tile([128, 4, BS], bf16, tag="qT", bufs=2)
            for k in range(4):
                dma.dma_start(out=qT[:, k, :], in_=q_sb[:, k * 128:(k + 1) * 128],
                              transpose=True)
            return qT

        def phase_att(s, g, qT):
            gsl_b = slice(g * GB, (g + 1) * GB)
            th = big.tile([128, 4, T * GB], bf16, tag=f"th{g}", name=f"th{g}_{s}")
            pe = peg[g].tile([128, 512], f32, tag=f"pe{g}")
            for k in range(4):
                nc.vector.tensor_add(
                    th[:, k, :].rearrange("p (t b) -> p t b", t=T),
                    hprojT[g][:, k, :].rearrange("p (t b) -> p t b", t=T),
                    bcast_t(qT[:, k, gsl_b]))
                nc.scalar.activation(out=th[:, k, :], in_=th[:, k, :], func=AF.Tanh)
                for j in range(NCH):
                    bp = 32 * j
                    nc.tensor.matmul(pe[bp:bp + 32, :], ws[:, k, :],
                                     th[:, k, j * 512:(j + 1) * 512],
                                     start=(k == 0), stop=(k == 3),
                                     tile_position=(0, bp))
            est = small.tile([128, 512], f32, tag=f"est{g}", bufs=1,
                             name=f"est{g}_{s}")
            nc.vector.tensor_copy(est, pe)
            est_ap = est[:]
            src = _b.AP(tensor=est_ap.tensor, offset=est_ap.offset,
                        ap=[[est_ap.ap[0][0] * 32, 4], est_ap.ap[1]])
            dma.dma_start(out=escr_d[g, s, :], in_=src)
            e_sb = small.tile([GB, T], f32, tag=f"e_sb{g}", bufs=1,
                              name=f"e_sb{g}_{s}")
            esl = escr_d[g, s, :]
            src2 = _b.AP(tensor=esl.tensor, offset=esl.offset,
                         ap=[[1, GB], [GB, T]])
            dma.dma_start(out=e_sb, in_=src2)
            return e_sb

        def phase_post(s, g, e_sb, ctx_sb):
            # softmax over t, alpha scatter, ctx matmul, copy into joint ctx_sb
            mx = tiny.tile([GB, 1], f32, tag=f"mx{g}")
            nc.vector.reduce_max(mx, e_sb, axis=AX.X)
            nmx = tiny.tile([GB, 1], f32, tag=f"nmx{g}")
            nc.vector.tensor_scalar_mul(nmx, mx, -1.0)
            ex = small.tile([GB, T], f32, tag=f"ex{g}", bufs=1, name=f"ex{g}_{s}")
            nc.scalar.activation(out=ex, in_=e_sb, func=AF.Exp, bias=nmx, scale=1.0)
            sm = tiny.tile([GB, 1], f32, tag=f"sm{g}")
            nc.vector.reduce_sum(sm, ex, axis=AX.X)
            rcp = tiny.tile([GB, 1], f32, tag=f"rcp{g}")
            nc.vector.reciprocal(rcp, sm)
            nc.vector.tensor_scalar_mul(alpha_pad[g][:, 0:T], ex, rcp)
            alphaT = small.tile([128, GB], bf16, tag=f"alphaT{g}", bufs=2,
                                name=f"alphaT{g}_{s}")
            dma.dma_start(out=alphaT, in_=alpha_pad[g], transpose=True)
            aT = alphaT[:]
            ab = ablk[g][:]
            for par in (0, 1):
                srcp = _b.AP(tensor=aT.tensor, offset=aT.offset + par * aT.ap[1][0],
                             ap=[[aT.ap[0][0], T], [2 * aT.ap[1][0], GB // 2]])
                dst = _b.AP(tensor=ab.tensor,
                            offset=ab.offset + par * (64 * ab.ap[0][0] + ab.ap[2][0]),
                            ap=[[ab.ap[0][0], T], [ab.ap[1][0] + 2 * ab.ap[2][0], GB // 2]])
                dma.dma_start(out=dst, in_=srcp)
            pctx = pzg[g].tile([GB, C], f32, tag=f"pz{g}")
            for kt in range(GB // 2):
                nc.tensor.matmul(pctx, ablk[g][:, kt, :], bHc[g][:, kt, :],
                                 start=(kt == 0), stop=(kt == GB // 2 - 1))
            nc.scalar.copy(ctx_sb[g * GB:(g + 1) * GB, :], pctx)

        def phase_lstm(s, ctx_sb):
            # joint z for all 64 rows
            xTc = small.tile([128, 4, BS], bf16, tag="xTc", bufs=2,
                             name=f"xTc_{s}")
            for k in range(4):
                dma.dma_start(out=xTc[:, k, :], in_=ctx_sb[:, k * 128:(k + 1) * 128],
                              transpose=True)
            gate_sl = {"i": 0, "f": 1, "g": 2, "o": 3}
            sig = {}
            t1 = t2 = None
            for gname in ("f", "i", "g", "o"):
                zsl = slice(gate_sl[gname] * 512, (gate_sl[gname] + 1) * 512)
                pzt = pzj.tile([BS, 512], f32, tag="pzj")
                for k in range(4):
                    nc.tensor.matmul(pzt, xTc[:, k, :], kc[:, k, zsl],
                                     start=(k == 0), stop=False)
                for k in range(4):
                    nc.tensor.matmul(pzt, hTj[0][:, k, :], rr[:, k, zsl],
                                     start=False, stop=False)
                nc.tensor.matmul(pzt, oh[:, s, :], ko[:, zsl],
                                 start=False, stop=True)
                g_sb = gates.tile([BS, 512], f32, tag="gate", bufs=3)
                if gname == "g":
                    nc.scalar.activation(out=g_sb, in_=pzt, func=AF.Tanh)
                else:
                    nc.scalar.activation(out=g_sb, in_=pzt, func=AF.Tanh, scale=0.5)
                    nc.vector.tensor_scalar(out=g_sb, in0=g_sb,
                                            scalar1=0.5, scalar2=0.5,
                                            op0=mybir.AluOpType.mult,
                                            op1=mybir.AluOpType.add)
                sig[gname] = g_sb
                if gname == "f":
                    t1 = gates.tile([BS, H], f32, tag="tmp", bufs=2)
                    nc.vector.tensor_mul(t1, sig["f"], c_stj[0])
                elif gname == "g":
                    t2 = gates.tile([BS, H], f32, tag="tmp", bufs=2)
                    nc.vector.tensor_mul(t2, sig["i"], sig["g"])
                    c_stj[0] = state.tile([BS, H], f32, tag="c", name=f"c_{s}")
                    nc.vector.tensor_add(c_stj[0], t1, t2)
            tc_sb = gates.tile([BS, H], f32, tag="tmp", bufs=2)
            nc.scalar.activation(out=tc_sb, in_=c_stj[0], func=AF.Tanh)
            h_bf = small.tile([BS, H], bf16, tag="h_bf", bufs=1, name=f"h_bf_{s}")
            nc.vector.tensor_mul(h_bf, sig["o"], tc_sb)
            hTj[0] = state.tile([128, 4, BS], bf16, tag="hT", name=f"hT_{s}")
            for k in range(4):
                dma.dma_start(out=hTj[0][:, k, :],
                              in_=h_bf[:, k * 128:(k + 1) * 128], transpose=True)
            pp = peg[0].tile([128, 512], f32, tag="pe0")
            for k in range(4):
                nc.tensor.matmul(pp[0:BS, 0:NCC], hTj[0][:, k, :], wg[:, k, :],
                                 start=(k == 0), stop=(k == 3))
            pr_sb = small.tile([BS, NCC], f32, tag="pr_sb", bufs=2,
                               name=f"pr_{s}")
            nc.vector.tensor_add(pr_sb, pp[0:BS, 0:NCC], bg)
            dma.dma_start(out=out_d[:, s, :], in_=pr_sb)

        for s in range(S):
            qT = phase_q(s)
            e0 = phase_att(s, 0, qT)
            e1 = phase_att(s, 1, qT)
            ctx_sb = small.tile([BS, C], bf16, tag="ctx_sb", bufs=2,
                                name=f"ctx_{s}")
            phase_post(s, 0, e0, ctx_sb)
            phase_post(s, 1, e1, ctx_sb)
            phase_lstm(s, ctx_sb)

    nc.finalize()
    return nc


def _prep_core(inputs, i):
    bsl = slice(i * BS, (i + 1) * BS)
    bh_i = np.asarray(inputs["batch_H"][bsl], np.float32)          # [64, 64, 512]
    text_i = np.asarray(inputs["text"][bsl])                       # [64, 26]
    bh_g = bh_i.reshape(NG, GB, T, C)
    m = {}
    m["bHT"] = np.ascontiguousarray(bh_g.transpose(0, 3, 2, 1)).astype(BF)
    m["bHc"] = np.ascontiguousarray(bh_g.reshape(NG, GB // 2, 128, C)).astype(BF)
    m["wi"] = np.asarray(inputs["Wi"], np.float32).astype(BF)
    m["wh"] = np.asarray(inputs["Wh"], np.float32).astype(BF)
    m["bh"] = np.ascontiguousarray(
        np.asarray(inputs["bh"], np.float32).reshape(4, 128).T)
    wsr = np.ascontiguousarray(
        np.asarray(inputs["Ws"], np.float32)[:, 0].reshape(4, 128).T).astype(BF)
    m["ws"] = np.repeat(wsr[:, :, None], 32, axis=2)
    lk = np.asarray(inputs["lstm_kernel"], np.float32)
    lb = np.asarray(inputs["lstm_bias"], np.float32)
    m["kc"] = lk[:C].astype(BF)
    m["ko"] = (lk[C:] + lb[None, :]).astype(BF)
    m["rr"] = np.asarray(inputs["lstm_rec"], np.float32).astype(BF)
    m["oh"] = (np.arange(NCC)[:, None, None] == text_i.T[None, :, :]).astype(BF)
    m["wg"] = np.asarray(inputs["Wgen"], np.float32).astype(BF)
    m["bg"] = np.tile(np.asarray(inputs["bgen"], np.float32)[None, :], (BS, 1))
    return m


def kernel(_trace=False, **inputs):
    from concourse import bass_utils
    if "nc" not in _CACHE:
        _CACHE["nc"] = build_bass()
    nc = _CACHE["nc"]
    in_maps = [_prep_core(inputs, i) for i in range(NCORES)]
    res = bass_utils.run_bass_kernel_spmd(nc, in_maps, list(range(NCORES)),
                                          trace=_trace)
    _CACHE["last_result"] = res
    out = np.concatenate([r["out"] for r in res.results], axis=0)
    return out.astype(np.float32)

